# revision 1
# baseline (speedup 1.0000x reference)
"""Trainium2 Bass kernel for nn_DynamLinear: per-codebook linear -> chunked
outer product -> mean over codebooks -> RMS norm.

Math notes:
  ref: y = einsum('td,hdo->tho', x, W); split o=64 into a=y[..., :32], b=y[..., 32:]
       op[t,h,i,j] = a[t,h,i]*b[t,h,j];  out = mean_h(op)*sqrt(16); rms_norm(out)
  Since rms_norm is scale invariant, out = S / sqrt(mean(S^2) + 16e-12) where
       S[t,i,j] = sum_h a[t,h,i]*b[t,h,j]  (the per-token 16x32^T @ 16x32 matmul)

Per-core plan (tokens sharded 1024/core):
  stage1: y^T = Wp^T @ x^T on TensorE (bf16), columns ordered so that a
          SBUF->SBUF DMA "shuffle" lands y into z[32r+h, sel, i, t256]
          (r = token/256, sel = a/b, t256 = token%256).
  stage2: per token one self-loading matmul lhsT=A_t[16h x 32i],
          rhs=B_t[16h x 32j] on a 32x32 PE tile (row group r, col group
          c = token%4) -> PSUM S_t[32i x 32j].
  rms:    ACT square, DVE reduce over j, indicator-matmul reduces over i
          (and broadcasts the per-token sums to all 128 partitions),
          sqrt+reciprocal, DVE multiply, strided DMA store.
"""

import os
import sys
import functools
from contextlib import ExitStack

import numpy as np
import ml_dtypes

if "/opt/trn_rl_repo" not in sys.path:
    sys.path.insert(0, "/opt/trn_rl_repo")

import concourse.bass as bass
import concourse.bacc as bacc
import concourse.tile as tile
from concourse import mybir
from concourse.bass_utils import run_bass_kernel_spmd

N_CORES = 8
T_CORE = 1024          # tokens per core
D = 1024               # feat dim
H = 16                 # codebooks
EPS = 16e-12           # 16 * 1e-12 (scale-folded reference eps)

F32 = mybir.dt.float32
F32R = mybir.dt.float32r
BF16 = mybir.dt.bfloat16


def _kernel_body(tc, ctx, xt, wp, ind, out):
    nc = tc.nc

    singles = ctx.enter_context(tc.tile_pool(name="singles", bufs=1))
    psum1 = ctx.enter_context(tc.tile_pool(name="psum1", bufs=3, space="PSUM"))
    psum2 = ctx.enter_context(tc.tile_pool(name="psum2", bufs=4, space="PSUM"))
    psum3 = ctx.enter_context(tc.tile_pool(name="psum3", bufs=1, space="PSUM"))
    scratch = ctx.enter_context(tc.tile_pool(name="scratch", bufs=2))
    smalls = ctx.enter_context(tc.tile_pool(name="smalls", bufs=4))

    # ---- resident inputs -------------------------------------------------
    wp_sb = singles.tile([128, 8, 8, 128], BF16)  # [dp, CT, dt, c7]
    xt_sb = singles.tile([128, 8, 1024], BF16)    # [dp, dt, t]: x^T
    nc.sync.dma_start(out=wp_sb[:, 0], in_=wp[:, 0])
    nc.sync.dma_start(out=xt_sb[:, :, 0:256], in_=xt[:, :, 0:256])
    for CT in range(1, 8):
        nc.sync.dma_start(out=wp_sb[:, CT], in_=wp[:, CT])
    for q in range(1, 4):
        nc.sync.dma_start(out=xt_sb[:, :, q * 256:(q + 1) * 256],
                          in_=xt[:, :, q * 256:(q + 1) * 256])
    ind_sb = singles.tile([128, 128], F32R)       # block-diag ones (4x 32x32)
    nc.sync.dma_start(out=ind_sb[:], in_=ind[:])
    eps_sb = singles.tile([128, 1], F32)
    nc.vector.memset(eps_sb[:], EPS)

    # ---- pipelined over token quarters tq (= row group r) ----------------
    # stage 1: y^T[col, t] for quarter tq -> shuffle(tq) -> stage2 chunks
    y_sb = singles.tile([128, 8, 1024], BF16)    # [p, CT, t]
    z4 = singles.tile([128, 2, 4, 8, 256], BF16)
    zout = singles.tile([128, 16, 16, 32], F32)  # [part, chunk, t16, j]
    out4 = out.rearrange("p (ch f) -> p ch f", ch=16)
    def _stage1(tq):
        # ---- stage 1 for this quarter (all col tiles) --------------------
        t0q = tq * 256
        for CT in range(8):
            ps = psum1.tile([128, 256], F32)
            for d in range(8):
                nc.tensor.matmul(
                    ps[:],
                    lhsT=wp_sb[:, CT, d, :],
                    rhs=xt_sb[:, d, t0q:t0q + 256],
                    start=(d == 0),
                    stop=(d == 7),
                )
            nc.scalar.activation(
                y_sb[:, CT, t0q:t0q + 256], ps[:],
                mybir.ActivationFunctionType.Copy,
            )
        # ---- shuffle this quarter into z row group tq --------------------
        r = tq
        for m in range(8):
            eng = nc.sync if (m % 2 == 0) else nc.gpsimd
            eng.dma_start(
                out=z4[32 * r:32 * r + 16, :, :, m, :],
                in_=y_sb[16 * m:16 * m + 16, :, t0q:t0q + 256],
            )

        # ---- stage 2 + rms for the two 128-token chunks of this quarter --
    def _stage2(tq):
        r = tq
        for half in range(4):
            ch = 4 * tq + half
            t0 = 64 * half
            ps2 = psum2.tile([128, 16, 32], F32)
            for tw in range(64):
                c, t32 = tw % 4, tw // 4
                t256 = t0 + tw
                nc.tensor.matmul(
                    ps2[32 * c:32 * c + 32, t32, :],
                    lhsT=z4[32 * r:32 * r + 16, 0, :, :, t256],
                    rhs=z4[32 * r:32 * r + 16, 1, :, :, t256],
                    start=True, stop=True,
                    tile_position=(32 * r, 32 * c),
                )
            sq = scratch.tile([128, 16, 32], F32)
            nc.scalar.square(sq[:], ps2[:])
            part = smalls.tile([128, 16], F32R)
            with nc.allow_low_precision(reason="f32r sum of 32 sq for rms"):
                nc.vector.tensor_reduce(part[:], sq[:],
                                        axis=mybir.AxisListType.X,
                                        op=mybir.AluOpType.add)
            ps3 = psum3.tile([128, 16], F32)
            nc.tensor.matmul(ps3[:], lhsT=ind_sb[:], rhs=part[:],
                             start=True, stop=True)
            s_sb = smalls.tile([128, 16], F32)
            nc.scalar.activation(s_sb[:], ps3[:],
                                 mybir.ActivationFunctionType.Sqrt,
                                 bias=eps_sb[:], scale=1.0 / 1024.0)
            rstd = smalls.tile([128, 16], F32)
            nc.vector.reciprocal(rstd[:], s_sb[:])
            nc.vector.tensor_mul(zout[:, ch], ps2[:],
                                 rstd[:].unsqueeze(2).broadcast_to([128, 16, 32]))
            # store: device layout [32c+i, ch, t32, j]; host unpermutes
            nc.sync.dma_start(
                out=out4[:, ch, :],
                in_=zout[:, ch].rearrange("p a b -> p (a b)"),
            )



    for tq in range(4):
        _stage1(tq)
        if tq > 0:
            _stage2(tq - 1)
    _stage2(3)

@functools.lru_cache(maxsize=1)
def _build_program():
    nc = bacc.Bacc("TRN2", target_bir_lowering=False, debug=False)
    xt = nc.dram_tensor("xt", [128, 8, 1024], BF16, kind="ExternalInput").ap()
    wp = nc.dram_tensor("wp", [128, 8, 8, 128], BF16, kind="ExternalInput").ap()
    ind = nc.dram_tensor("ind", [128, 128], F32R, kind="ExternalInput").ap()
    out = nc.dram_tensor("out", [128, 8192], F32, kind="ExternalOutput").ap()
    with tile.TileContext(nc) as tc:
        with ExitStack() as ctx:
            _kernel_body(tc, ctx, xt, wp, ind, out)
    nc.compile()
    return nc


def _host_prep(x, weight):
    xf = np.ascontiguousarray(x.reshape(-1, D))          # [8192, 1024]
    # Wp column order: col = 512*sel + 128*ctp + 16*m + h ; i = 8*ctp + m
    w = weight.transpose(1, 0, 2).reshape(D, H, 2, 4, 8)  # [d, h, sel, ctp, m]
    wp = w.transpose(0, 2, 3, 4, 1).reshape(D, 1024)      # [d, col]
    wp_sb = np.ascontiguousarray(
        wp.reshape(8, 128, 8, 128).transpose(1, 2, 0, 3)).astype(
            ml_dtypes.bfloat16)
    ind = np.kron(np.eye(4, dtype=np.float32),
                  np.ones((32, 32), dtype=np.float32))
    xt_shards = []
    for c in range(N_CORES):
        xt = xf[c * T_CORE:(c + 1) * T_CORE].T            # [d, t]
        xt_sb = np.ascontiguousarray(
            xt.reshape(8, 128, 1024).transpose(1, 0, 2)).astype(
                ml_dtypes.bfloat16)
        xt_shards.append(xt_sb)
    return xt_shards, wp_sb, ind


def kernel(x, weight, **_unused):
    x = np.asarray(x, dtype=np.float32)
    weight = np.asarray(weight, dtype=np.float32)
    xt_shards, wp_sb, ind = _host_prep(x, weight)
    nc = _build_program()
    in_maps = [{"xt": xt_shards[c], "wp": wp_sb, "ind": ind}
               for c in range(N_CORES)]
    res = run_bass_kernel_spmd(nc, in_maps, list(range(N_CORES)))
    outs = []
    for c in range(N_CORES):
        d = np.asarray(res.results[c]["out"]).reshape(4, 32, 16, 16, 32)
        # [cg, i, ch, t32, j] -> token t = 128*ch + 4*t32 + cg, row = i*32+j
        outs.append(d.transpose(2, 3, 0, 1, 4).reshape(T_CORE, 1024))
    full = np.concatenate(outs, axis=0)                   # [8192, 1024]
    return full.reshape(x.shape[0], x.shape[1], 1024).astype(np.float32)


if __name__ == "__main__":
    rng = np.random.default_rng(0)
    x = rng.standard_normal((4, 2048, D), dtype=np.float32)
    w = (rng.standard_normal((H, D, 64), dtype=np.float32)
         * np.sqrt(2.0 / (D + 64))).astype(np.float32)
    o = kernel(x, w)
    print(o.shape, o.dtype)



# revision 2
# speedup vs baseline: 1.0363x; 1.0363x over previous
"""Trainium2 Bass kernel for nn_DynamLinear, v2.

Math (see reference.py): y = einsum('td,hdo->tho', x, W); a = y[...,:32],
b = y[...,32:]; S[t] = sum_h a[t,h,:] (x) b[t,h,:]  (16x32^T @ 16x32);
out = S / sqrt(mean_ij(S^2) + 16e-12)   (rms_norm is scale invariant).

Per-core plan (tokens sharded 1024/core, t = 512*H + 256*v + g):
  stage1: y^T = Wp^T @ x^T on TensorE (bf16), psum [128 cols, 2, 512 t] per
          (col-tile pair, half H); ACT/DVE/Pool copy -> y_sb[col, t] bf16.
  shuffle (through HBM): store y_sb -> ybuf[row = 512*sel+128*c+16*m+h, t]
          (4 quarter-stores x 2 sel, partition-uniform APs), then load
          ab[16*v+h, slot, g] <- ybuf with slot = 32*v+i for A (i = 8c+m)
          and 64+j for B.  Off-block A slots are pre-zeroed once from HBM.
          Load out-APs are emitted slot-major ([32 slot, 16 part, 256 g])
          so the cost model's per-queue charge reflects the actual 512B
          descriptor stream instead of a 16-partition-wide transfer.
  stage2: per group g one matmul lhsT = ab[0:32, 0:64, g] (block-diag
          A_t0/A_t1), rhs = ab[0:32, 64:96, g] -> S for 2 tokens at psum
          partitions [0:64); a second matmul at tile column 64 packs 2 more
          tokens -> full [128, 16, 32] psum chunks for full-width RMS.
  rms:    square (ACT), reduce_j (DVE 2x bf16), block-ones matmul reduces i
          and broadcasts (PE), Rsqrt (ACT), multiply (DVE/Pool), bf16 out.
"""

import sys
import functools
from contextlib import ExitStack

import numpy as np
import ml_dtypes

if "/opt/trn_rl_repo" not in sys.path:
    sys.path.insert(0, "/opt/trn_rl_repo")

import concourse.bass as bass
import concourse.bacc as bacc
import concourse.tile as tile
from concourse import mybir
from concourse.bass_utils import run_bass_kernel_spmd

N_CORES = 8
T_CORE = 1024          # tokens per core
D = 1024               # feat dim
H = 16                 # codebooks
EPS = 16e-12           # 16 * 1e-12 (scale-folded reference eps)

F32 = mybir.dt.float32
BF16 = mybir.dt.bfloat16
AF = mybir.ActivationFunctionType


def _kernel_body(tc, ctx, xt, wp, ind, zz, ybuf, out):
    nc = tc.nc

    singles = ctx.enter_context(tc.tile_pool(name="singles", bufs=1))
    py = ctx.enter_context(tc.tile_pool(name="py", bufs=3, space="PSUM"))
    ps2p = ctx.enter_context(tc.tile_pool(name="ps2", bufs=4, space="PSUM"))
    ps3p = ctx.enter_context(tc.tile_pool(name="ps3", bufs=1, space="PSUM"))
    ps3_all = ps3p.tile([128, 16, 16], F32)
    sqp = ctx.enter_context(tc.tile_pool(name="sqp", bufs=3))
    smalls = ctx.enter_context(tc.tile_pool(name="smalls", bufs=6))

    # ---- resident SBUF tensors ------------------------------------------
    wp_sb = singles.tile([128, 8, 8, 128], BF16)   # [dp, CT, dt, col]
    xt_sb = singles.tile([128, 8, 1024], BF16)     # [dp, dt, t]
    y_sb = singles.tile([128, 8, 1024], BF16)      # [colp, CT, t]
    # slots: A-H0 [0:64), A-H1 [64:128), B-H0 [128:160), B-H1 [160:192)
    ab = singles.tile([32, 192, 256], BF16)        # [16v+h, slot, g]
    ind_sb = singles.tile([128, 128], BF16)        # 4x (32x32 ones) blk diag
    zout = singles.tile([128, 2, 8, 512], BF16)    # [p, H, ch, (w j)]
    eps_sb = singles.tile([128, 1], F32)

    nc.sync.dma_start(out=xt_sb[:, 0:2, 0:512], in_=xt[:, 0:2, 0:512])
    nc.scalar.dma_start(out=wp_sb[:, 0:1], in_=wp[:, 0:1])
    nc.scalar.dma_start(out=wp_sb[:, 1:2], in_=wp[:, 1:2])
    nc.sync.dma_start(out=xt_sb[:, 2:5, 0:512], in_=xt[:, 2:5, 0:512])
    nc.sync.dma_start(out=xt_sb[:, 5:8, 0:512], in_=xt[:, 5:8, 0:512])
    nc.scalar.dma_start(out=wp_sb[:, 2:4], in_=wp[:, 2:4])
    nc.sync.dma_start(out=xt_sb[:, :, 512:1024], in_=xt[:, :, 512:1024])
    nc.scalar.dma_start(out=wp_sb[:, 4:8], in_=wp[:, 4:8])
    nc.gpsimd.dma_start(out=ind_sb[:], in_=ind[:])
    # zero the off-block A slots (both A buffers) from the zz input
    zz3 = zz[:].rearrange("p (s g) -> p s g", s=32)
    zeng = [nc.gpsimd, nc.gpsimd, nc.gpsimd, nc.sync,
            nc.gpsimd, nc.gpsimd, nc.sync, nc.gpsimd]
    zi = 0
    for Hh in range(2):
        a0 = 64 * Hh
        for (p0, s0) in ((0, 32), (16, 0)):
            for half in range(2):
                zeng[zi].dma_start(
                    out=ab[p0:p0 + 16, a0 + s0 + 16 * half:
                           a0 + s0 + 16 * half + 16, :],
                    in_=zz3[:, 16 * half:16 * half + 16, :])
                zi += 1
    nc.vector.memset(eps_sb[:], EPS)

    # views of ybuf [1024 rows = (sel, c, p=8h+m), 1024 t]
    yb_st = ybuf.rearrange("(sel c p) t -> sel p c t", sel=2, c=4)
    yb_a = ybuf.rearrange("(sel c h m) t -> sel c h m t", sel=2, c=4, h=16)
    yb_b = ybuf.rearrange("(sel c p) t -> sel c p t", sel=2, c=4)

    def _copy(dst, ps, e):
        if e == "act":
            nc.scalar.activation(dst, ps, AF.Copy)
        elif e == "dve":
            nc.vector.tensor_copy(dst, ps)
        else:
            nc.gpsimd.tensor_copy(dst, ps)

    def stage1_pair(Hh, pr, eng):
        t0 = 512 * Hh
        for u in range(2):
            CT = 2 * pr + u
            ps = py.tile([128, 512], F32)
            for d in range(8):
                nc.tensor.matmul(
                    ps[:],
                    lhsT=wp_sb[:, CT, d, :],
                    rhs=xt_sb[:, d, t0:t0 + 512],
                    start=(d == 0),
                    stop=(d == 7),
                )
            e = eng if u == 0 else eng2[eng]
            _copy(y_sb[:, CT, t0:t0 + 512], ps[:], e)

    def store_q(q, sel, eng):
        eng.dma_start(
            out=yb_st[sel][:, :, 256 * q:256 * q + 256],
            in_=y_sb[:, 4 * sel:4 * sel + 4, 256 * q:256 * q + 256],
        )

    def load_a(Hh, v, c, eng):
        # block-diag A piece: ab[16v+h, 64H+32v+8c+m, g] <- ybuf(A, c, h, m)
        q = 2 * Hh + v
        s0 = 64 * Hh + 32 * v + 8 * c
        eng.dma_start(
            out=ab[16 * v:16 * v + 16, s0:s0 + 8, :],
            in_=yb_a[0, c][:, :, 256 * q:256 * q + 256],
        )

    def load_b(Hh, c, eng):
        # stacked B piece: ab[16v+h (all 32), 128+32H+8c+m, g] <- ybuf(B, c)
        s0 = 128 + 32 * Hh + 8 * c
        eng.dma_start(
            out=ab[0:32, s0:s0 + 8, :],
            in_=yb_b[1, c][:, 512 * Hh:512 * Hh + 512].rearrange(
                "p (v g) -> v p g", v=2),
        )

    def stage2_chunk(Hh, ch, mul_eng, sq_eng=None, w0=0, nw=16):
        ps2 = ps2p.tile([128, nw, 32], F32)
        for w in range(nw):
            for k in range(2):
                g = 32 * ch + 16 * k + w0 + w
                nc.tensor.matmul(
                    ps2[64 * k:64 * k + 64, w, :],
                    lhsT=ab[0:32, 64 * Hh:64 * Hh + 64, g],
                    rhs=ab[0:32, 128 + 32 * Hh:160 + 32 * Hh, g],
                    start=True, stop=True,
                )
        sq = sqp.tile([128, nw, 32], BF16)
        if sq_eng is None:
            nc.scalar.square(sq[:], ps2[:])
        else:
            sq_eng.tensor_mul(sq[:], ps2[:], ps2[:])
        part = smalls.tile([128, nw], BF16)
        with nc.allow_low_precision(reason="bf16 rms partial sums"):
            nc.vector.tensor_reduce(part[:], sq[:],
                                    axis=mybir.AxisListType.X,
                                    op=mybir.AluOpType.add)
        ps3 = ps3_all[:, 8 * Hh + ch, 0:nw]
        nc.tensor.matmul(ps3, lhsT=ind_sb[:], rhs=part[:],
                         start=True, stop=True)
        s_sb = smalls.tile([128, nw], F32)
        nc.scalar.activation(s_sb[:], ps3, AF.Sqrt,
                             bias=eps_sb[:], scale=1.0 / 1024.0)
        rstd = smalls.tile([128, nw], F32)
        nc.vector.reciprocal(rstd[:], s_sb[:])
        zv = zout[:, Hh, ch].rearrange("p (w j) -> p w j", w=16)[:, w0:w0 + nw]
        mul_eng.tensor_mul(zv, ps2[:],
                           rstd[:].unsqueeze(2).broadcast_to([128, nw, 32]))

    def store_out(Hh, c0, eng, n=4):
        eng.dma_start(out=out[:, Hh, c0:c0 + n, :],
                      in_=zout[:, Hh, c0:c0 + n, :])

    mul_engs_unused = None
    copy_engs = ["dve", "dve", "dve", "act", "dve", "dve", "dve", "dve"]
    eng2 = {"dve": "act", "act": "dve", "pool": "dve"}
    mul_engs = [nc.vector] * 16

    # ================= schedule =================
    # ---- half 0 stage 1 ----
    stage1_pair(0, 0, copy_engs[0])
    stage1_pair(0, 1, copy_engs[1])
    store_q(0, 0, nc.sync)
    store_q(1, 0, nc.sync)
    for c in range(4):
        load_a(0, 0, c, nc.sync if c % 2 == 0 else nc.scalar)
        load_a(0, 1, c, nc.scalar if c % 2 == 0 else nc.sync)
    stage1_pair(0, 2, copy_engs[2])
    stage1_pair(0, 3, copy_engs[3])
    store_q(0, 1, nc.sync)
    store_q(1, 1, nc.scalar)
    for c in range(4):
        load_b(0, c, [nc.sync, nc.scalar, nc.gpsimd, nc.sync][c])

    # ---- half 1 stage 1, interleaved with half-0 stage 2 ----
    stage1_pair(1, 0, copy_engs[4])
    stage1_pair(1, 1, copy_engs[5])
    store_q(2, 0, nc.sync)
    store_q(3, 0, nc.sync)
    for c in range(4):
        load_a(1, 0, c, nc.sync if c % 2 == 0 else nc.gpsimd)
        load_a(1, 1, c, nc.gpsimd if c % 2 == 0 else nc.sync)
    for ch in range(4):
        stage2_chunk(0, ch, mul_engs[ch])
    stage1_pair(1, 2, copy_engs[6])
    stage1_pair(1, 3, copy_engs[7])
    store_q(2, 1, nc.sync)
    store_q(3, 1, nc.scalar)
    for c in range(4):
        load_b(1, c, [nc.sync, nc.gpsimd, nc.scalar, nc.sync][c])
    for ch in range(4, 8):
        stage2_chunk(0, ch, mul_engs[ch])
    store_out(0, 0, nc.sync)
    store_out(0, 4, nc.sync)

    # ---- half 1 stage 2 ----
    h1_muls = [nc.vector] * 8
    for ch in range(8):
        stage2_chunk(1, ch, h1_muls[ch])
        if ch == 3:
            store_out(1, 0, nc.sync)
        if ch == 5:
            store_out(1, 4, nc.sync, n=2)
        if ch == 7:
            store_out(1, 6, nc.sync, n=2)


def build_program(trace_sim=False):
    nc = bacc.Bacc("TRN2", target_bir_lowering=False, debug=False)
    xt = nc.dram_tensor("xt", [128, 8, 1024], BF16, kind="ExternalInput").ap()
    wp = nc.dram_tensor("wp", [128, 8, 8, 128], BF16, kind="ExternalInput").ap()
    ind = nc.dram_tensor("ind", [128, 128], BF16, kind="ExternalInput").ap()
    zz = nc.dram_tensor("zz", [16, 8192], BF16, kind="ExternalInput").ap()
    ybuf = nc.dram_tensor("ybuf", [1024, 1024], BF16, kind="Internal").ap()
    out = nc.dram_tensor("out", [128, 2, 8, 512], BF16,
                         kind="ExternalOutput").ap()
    with tile.TileContext(nc, trace_sim=trace_sim) as tc:
        with ExitStack() as ctx:
            _kernel_body(tc, ctx, xt, wp, ind, zz, ybuf, out)
    if not trace_sim:
        nc.compile()
    return nc


@functools.lru_cache(maxsize=1)
def _built_program():
    return build_program(trace_sim=False)


def _host_prep(x, weight):
    xf = np.ascontiguousarray(x.reshape(-1, D))          # [8192, 1024]
    # Wp column order: col = 512*sel + 128*c + 8*h + m ; i = 8*c + m
    w = weight.transpose(1, 0, 2).reshape(D, H, 2, 4, 8)  # [d, h, sel, c, m]
    wp = w.transpose(0, 2, 3, 1, 4).reshape(D, 1024)      # [d, col]
    wp_sb = np.ascontiguousarray(
        wp.reshape(8, 128, 8, 128).transpose(1, 2, 0, 3)).astype(
            ml_dtypes.bfloat16)
    ind = np.kron(np.eye(4, dtype=np.float32),
                  np.ones((32, 32), dtype=np.float32)).astype(ml_dtypes.bfloat16)
    zz = np.zeros((16, 8192), dtype=ml_dtypes.bfloat16)
    xt_shards = []
    for c in range(N_CORES):
        xtc = xf[c * T_CORE:(c + 1) * T_CORE].T            # [d, t]
        xt_sb = np.ascontiguousarray(
            xtc.reshape(8, 128, 1024).transpose(1, 0, 2)).astype(
                ml_dtypes.bfloat16)
        xt_shards.append(xt_sb)
    return xt_shards, wp_sb, ind, zz


def kernel(x, weight, **_unused):
    x = np.asarray(x, dtype=np.float32)
    weight = np.asarray(weight, dtype=np.float32)
    xt_shards, wp_sb, ind, zz = _host_prep(x, weight)
    nc = _built_program()
    in_maps = [{"xt": xt_shards[c], "wp": wp_sb, "ind": ind, "zz": zz}
               for c in range(N_CORES)]
    res = run_bass_kernel_spmd(nc, in_maps, list(range(N_CORES)))
    outs = []
    for c in range(N_CORES):
        d = np.asarray(res.results[c]["out"]).astype(np.float32)
        d = d.reshape(2, 2, 32, 2, 8, 16, 32)  # [k, v, i, H, ch, w, j]
        # token t = 512H + 256v + 32ch + 16k + w ; element (i, j)
        o = d.transpose(3, 1, 4, 0, 5, 2, 6).reshape(T_CORE, 1024)
        outs.append(o)
    full = np.concatenate(outs, axis=0)                   # [8192, 1024]
    return full.reshape(x.shape[0], x.shape[1], 1024).astype(np.float32)


if __name__ == "__main__":
    rng = np.random.default_rng(0)
    x = rng.standard_normal((4, 2048, D), dtype=np.float32)
    w = (rng.standard_normal((H, D, 64), dtype=np.float32)
         * np.sqrt(2.0 / (D + 64))).astype(np.float32)
    o = kernel(x, w)
    print(o.shape, o.dtype)


# revision 3
# speedup vs baseline: 1.0671x; 1.0297x over previous
"""Trainium2 Bass kernel for nn_DynamLinear, v2.

Math (see reference.py): y = einsum('td,hdo->tho', x, W); a = y[...,:32],
b = y[...,32:]; S[t] = sum_h a[t,h,:] (x) b[t,h,:]  (16x32^T @ 16x32);
out = S / sqrt(mean_ij(S^2) + 16e-12)   (rms_norm is scale invariant).

Per-core plan (tokens sharded 1024/core, t = 512*H + 256*v + g):
  stage1: y^T = Wp^T @ x^T on TensorE (bf16), psum [128 cols, 2, 512 t] per
          (col-tile pair, half H); ACT/DVE/Pool copy -> y_sb[col, t] bf16.
  shuffle (through HBM): store y_sb -> ybuf[row = 512*sel+128*c+16*m+h, t]
          (4 quarter-stores x 2 sel, partition-uniform APs), then load
          ab[16*v+h, slot, g] <- ybuf with slot = 32*v+i for A (i = 8c+m)
          and 64+j for B.  Off-block A slots are pre-zeroed once from HBM.
          Load out-APs are emitted slot-major ([32 slot, 16 part, 256 g])
          so the cost model's per-queue charge reflects the actual 512B
          descriptor stream instead of a 16-partition-wide transfer.
  stage2: per group g one matmul lhsT = ab[0:32, 0:64, g] (block-diag
          A_t0/A_t1), rhs = ab[0:32, 64:96, g] -> S for 2 tokens at psum
          partitions [0:64); a second matmul at tile column 64 packs 2 more
          tokens -> full [128, 16, 32] psum chunks for full-width RMS.
  rms:    square (ACT), reduce_j (DVE 2x bf16), block-ones matmul reduces i
          and broadcasts (PE), Rsqrt (ACT), multiply (DVE/Pool), bf16 out.
"""

import sys
import functools
from contextlib import ExitStack

import numpy as np
import ml_dtypes

if "/opt/trn_rl_repo" not in sys.path:
    sys.path.insert(0, "/opt/trn_rl_repo")

import concourse.bass as bass
import concourse.bacc as bacc
import concourse.tile as tile
from concourse import mybir
from concourse.bass_utils import run_bass_kernel_spmd

N_CORES = 8
T_CORE = 1024          # tokens per core
D = 1024               # feat dim
H = 16                 # codebooks
EPS = 16e-12           # 16 * 1e-12 (scale-folded reference eps)

F32 = mybir.dt.float32
BF16 = mybir.dt.bfloat16
AF = mybir.ActivationFunctionType


def _kernel_body(tc, ctx, xt, wp, ind, zz, ybuf, out):
    nc = tc.nc

    singles = ctx.enter_context(tc.tile_pool(name="singles", bufs=1))
    py = ctx.enter_context(tc.tile_pool(name="py", bufs=3, space="PSUM"))
    ps2p = ctx.enter_context(tc.tile_pool(name="ps2", bufs=4, space="PSUM"))
    ps3p = ctx.enter_context(tc.tile_pool(name="ps3", bufs=1, space="PSUM"))
    ps3_all = ps3p.tile([128, 16, 16], F32)
    sqp = ctx.enter_context(tc.tile_pool(name="sqp", bufs=6))
    smalls = ctx.enter_context(tc.tile_pool(name="smalls", bufs=6))

    # ---- resident SBUF tensors ------------------------------------------
    wp_sb = singles.tile([128, 8, 8, 128], BF16)   # [dp, CT, dt, col]
    xt_sb = singles.tile([128, 8, 1024], BF16)     # [dp, dt, t]
    y_sb = singles.tile([128, 8, 1024], BF16)      # [colp, CT, t]
    # slots: A-H0 [0:64), A-H1 [64:128), B-H0 [128:160), B-H1 [160:192)
    ab = singles.tile([32, 192, 256], BF16)        # [16v+h, slot, g]
    ind_sb = singles.tile([128, 128], BF16)        # 4x (32x32 ones) blk diag
    zout = singles.tile([128, 2, 8, 512], BF16)    # [p, H, ch, (w j)]
    eps_sb = singles.tile([128, 1], F32)

    nc.sync.dma_start(out=xt_sb[:, 0:2, 0:512], in_=xt[:, 0:2, 0:512])
    nc.scalar.dma_start(out=wp_sb[:, 0:1], in_=wp[:, 0:1])
    nc.scalar.dma_start(out=wp_sb[:, 1:2], in_=wp[:, 1:2])
    nc.sync.dma_start(out=xt_sb[:, 2:5, 0:512], in_=xt[:, 2:5, 0:512])
    nc.sync.dma_start(out=xt_sb[:, 5:8, 0:512], in_=xt[:, 5:8, 0:512])
    nc.scalar.dma_start(out=wp_sb[:, 2:4], in_=wp[:, 2:4])
    nc.sync.dma_start(out=xt_sb[:, :, 512:1024], in_=xt[:, :, 512:1024])
    nc.scalar.dma_start(out=wp_sb[:, 4:8], in_=wp[:, 4:8])
    nc.gpsimd.dma_start(out=ind_sb[:], in_=ind[:])
    # zero the off-block A slots (both A buffers) from the zz input
    zz3 = zz[:].rearrange("p (s g) -> p s g", s=32)
    zeng = [nc.gpsimd, nc.gpsimd, nc.gpsimd, nc.sync,
            nc.gpsimd, nc.gpsimd, nc.sync, nc.gpsimd]
    zi = 0
    for Hh in range(2):
        a0 = 64 * Hh
        for (p0, s0) in ((0, 32), (16, 0)):
            for half in range(2):
                zeng[zi].dma_start(
                    out=ab[p0:p0 + 16, a0 + s0 + 16 * half:
                           a0 + s0 + 16 * half + 16, :],
                    in_=zz3[:, 16 * half:16 * half + 16, :])
                zi += 1
    nc.vector.memset(eps_sb[:], EPS)

    # views of ybuf [1024 rows = (sel, c, p=8h+m), 1024 t]
    yb_st = ybuf.rearrange("(sel c p) t -> sel p c t", sel=2, c=4)
    yb_a = ybuf.rearrange("(sel c h m) t -> sel c h m t", sel=2, c=4, h=16)
    yb_b = ybuf.rearrange("(sel c p) t -> sel c p t", sel=2, c=4)

    def _copy(dst, ps, e):
        if e == "act":
            nc.scalar.activation(dst, ps, AF.Copy)
        elif e == "dve":
            nc.vector.tensor_copy(dst, ps)
        else:
            nc.gpsimd.tensor_copy(dst, ps)

    def stage1_pair(Hh, pr, eng):
        t0 = 512 * Hh
        for u in range(2):
            CT = 2 * pr + u
            ps = py.tile([128, 512], F32)
            for d in range(8):
                nc.tensor.matmul(
                    ps[:],
                    lhsT=wp_sb[:, CT, d, :],
                    rhs=xt_sb[:, d, t0:t0 + 512],
                    start=(d == 0),
                    stop=(d == 7),
                )
            e = eng if u == 0 else eng2[eng]
            _copy(y_sb[:, CT, t0:t0 + 512], ps[:], e)

    def store_q(q, sel, eng):
        eng.dma_start(
            out=yb_st[sel][:, :, 256 * q:256 * q + 256],
            in_=y_sb[:, 4 * sel:4 * sel + 4, 256 * q:256 * q + 256],
        )

    def load_a(Hh, v, c, eng):
        # block-diag A piece: ab[16v+h, 64H+32v+8c+m, g] <- ybuf(A, c, h, m)
        q = 2 * Hh + v
        s0 = 64 * Hh + 32 * v + 8 * c
        eng.dma_start(
            out=ab[16 * v:16 * v + 16, s0:s0 + 8, :],
            in_=yb_a[0, c][:, :, 256 * q:256 * q + 256],
        )

    def load_b(Hh, c, eng):
        # stacked B piece: ab[16v+h (all 32), 128+32H+8c+m, g] <- ybuf(B, c)
        s0 = 128 + 32 * Hh + 8 * c
        eng.dma_start(
            out=ab[0:32, s0:s0 + 8, :],
            in_=yb_b[1, c][:, 512 * Hh:512 * Hh + 512].rearrange(
                "p (v g) -> v p g", v=2),
        )

    def stage2_chunk(Hh, ch, cp_eng, w0=0, nw=16):
        ps2 = ps2p.tile([128, nw, 32], F32)
        for w in range(nw):
            for k in range(2):
                g = 32 * ch + 16 * k + w0 + w
                nc.tensor.matmul(
                    ps2[64 * k:64 * k + 64, w, :],
                    lhsT=ab[0:32, 64 * Hh:64 * Hh + 64, g],
                    rhs=ab[0:32, 128 + 32 * Hh:160 + 32 * Hh, g],
                    start=True, stop=True,
                )
        # copy S to SBUF so square and the final multiply can run on Pool
        s_sb = sqp.tile([128, nw, 32], BF16)
        if cp_eng == "act":
            nc.scalar.activation(s_sb[:], ps2[:], AF.Copy)
        else:
            nc.vector.tensor_copy(s_sb[:], ps2[:])
        sq = sqp.tile([128, nw, 32], BF16)
        nc.gpsimd.tensor_mul(sq[:], s_sb[:], s_sb[:])
        part = smalls.tile([128, nw], BF16)
        with nc.allow_low_precision(reason="bf16 rms partial sums"):
            nc.vector.tensor_reduce(part[:], sq[:],
                                    axis=mybir.AxisListType.X,
                                    op=mybir.AluOpType.add)
        ps3 = ps3_all[:, 8 * Hh + ch, 0:nw]
        nc.tensor.matmul(ps3, lhsT=ind_sb[:], rhs=part[:],
                         start=True, stop=True)
        s_rms = smalls.tile([128, nw], F32)
        nc.scalar.activation(s_rms[:], ps3, AF.Sqrt,
                             bias=eps_sb[:], scale=1.0 / 1024.0)
        rstd = smalls.tile([128, nw], F32)
        nc.vector.reciprocal(rstd[:], s_rms[:])
        zv = zout[:, Hh, ch].rearrange("p (w j) -> p w j", w=16)[:, w0:w0 + nw]
        nc.gpsimd.tensor_mul(zv, s_sb[:],
                             rstd[:].unsqueeze(2).broadcast_to([128, nw, 32]))

    def store_out(Hh, c0, eng, n=4):
        eng.dma_start(out=out[:, Hh, c0:c0 + n, :],
                      in_=zout[:, Hh, c0:c0 + n, :])

    mul_engs_unused = None
    copy_engs = ["dve", "dve", "dve", "act", "dve", "dve", "dve", "dve"]
    eng2 = {"dve": "act", "act": "dve", "pool": "dve"}
    mul_engs = [nc.vector] * 16

    # ================= schedule =================
    # ---- half 0 stage 1 ----
    stage1_pair(0, 0, copy_engs[0])
    stage1_pair(0, 1, copy_engs[1])
    store_q(0, 0, nc.sync)
    store_q(1, 0, nc.sync)
    for c in range(4):
        load_a(0, 0, c, nc.sync if c % 2 == 0 else nc.scalar)
        load_a(0, 1, c, nc.scalar if c % 2 == 0 else nc.sync)
    stage1_pair(0, 2, copy_engs[2])
    stage1_pair(0, 3, copy_engs[3])
    store_q(0, 1, nc.sync)
    store_q(1, 1, nc.scalar)
    for c in range(4):
        load_b(0, c, [nc.sync, nc.scalar, nc.gpsimd, nc.sync][c])

    # ---- half 1 stage 1, interleaved with half-0 stage 2 ----
    stage1_pair(1, 0, copy_engs[4])
    stage1_pair(1, 1, copy_engs[5])
    store_q(2, 0, nc.sync)
    store_q(3, 0, nc.sync)
    for c in range(4):
        load_a(1, 0, c, nc.sync if c % 2 == 0 else nc.gpsimd)
        load_a(1, 1, c, nc.gpsimd if c % 2 == 0 else nc.sync)
    for ch in range(4):
        stage2_chunk(0, ch, "act" if ch % 2 == 0 else "dve")
    stage1_pair(1, 2, copy_engs[6])
    stage1_pair(1, 3, copy_engs[7])
    store_q(2, 1, nc.sync)
    store_q(3, 1, nc.scalar)
    for c in range(4):
        load_b(1, c, [nc.sync, nc.gpsimd, nc.scalar, nc.sync][c])
    for ch in range(4, 8):
        stage2_chunk(0, ch, "act" if ch % 2 == 0 else "dve")
    store_out(0, 0, nc.sync)
    store_out(0, 4, nc.sync)

    # ---- half 1 stage 2 ----
    h1_muls = [nc.vector] * 8
    for ch in range(8):
        stage2_chunk(1, ch, "act" if ch % 2 == 0 else "dve")
        if ch == 3:
            store_out(1, 0, nc.sync)
        if ch == 5:
            store_out(1, 4, nc.sync, n=2)
        if ch == 7:
            store_out(1, 6, nc.sync, n=2)


def build_program(trace_sim=False):
    nc = bacc.Bacc("TRN2", target_bir_lowering=False, debug=False)
    xt = nc.dram_tensor("xt", [128, 8, 1024], BF16, kind="ExternalInput").ap()
    wp = nc.dram_tensor("wp", [128, 8, 8, 128], BF16, kind="ExternalInput").ap()
    ind = nc.dram_tensor("ind", [128, 128], BF16, kind="ExternalInput").ap()
    zz = nc.dram_tensor("zz", [16, 8192], BF16, kind="ExternalInput").ap()
    ybuf = nc.dram_tensor("ybuf", [1024, 1024], BF16, kind="Internal").ap()
    out = nc.dram_tensor("out", [128, 2, 8, 512], BF16,
                         kind="ExternalOutput").ap()
    with tile.TileContext(nc, trace_sim=trace_sim) as tc:
        with ExitStack() as ctx:
            _kernel_body(tc, ctx, xt, wp, ind, zz, ybuf, out)
    if not trace_sim:
        nc.compile()
    return nc


@functools.lru_cache(maxsize=1)
def _built_program():
    return build_program(trace_sim=False)


def _host_prep(x, weight):
    xf = np.ascontiguousarray(x.reshape(-1, D))          # [8192, 1024]
    # Wp column order: col = 512*sel + 128*c + 8*h + m ; i = 8*c + m
    w = weight.transpose(1, 0, 2).reshape(D, H, 2, 4, 8)  # [d, h, sel, c, m]
    wp = w.transpose(0, 2, 3, 1, 4).reshape(D, 1024)      # [d, col]
    wp_sb = np.ascontiguousarray(
        wp.reshape(8, 128, 8, 128).transpose(1, 2, 0, 3)).astype(
            ml_dtypes.bfloat16)
    ind = np.kron(np.eye(4, dtype=np.float32),
                  np.ones((32, 32), dtype=np.float32)).astype(ml_dtypes.bfloat16)
    zz = np.zeros((16, 8192), dtype=ml_dtypes.bfloat16)
    xt_shards = []
    for c in range(N_CORES):
        xtc = xf[c * T_CORE:(c + 1) * T_CORE].T            # [d, t]
        xt_sb = np.ascontiguousarray(
            xtc.reshape(8, 128, 1024).transpose(1, 0, 2)).astype(
                ml_dtypes.bfloat16)
        xt_shards.append(xt_sb)
    return xt_shards, wp_sb, ind, zz


def kernel(x, weight, **_unused):
    x = np.asarray(x, dtype=np.float32)
    weight = np.asarray(weight, dtype=np.float32)
    xt_shards, wp_sb, ind, zz = _host_prep(x, weight)
    nc = _built_program()
    in_maps = [{"xt": xt_shards[c], "wp": wp_sb, "ind": ind, "zz": zz}
               for c in range(N_CORES)]
    res = run_bass_kernel_spmd(nc, in_maps, list(range(N_CORES)))
    outs = []
    for c in range(N_CORES):
        d = np.asarray(res.results[c]["out"]).astype(np.float32)
        d = d.reshape(2, 2, 32, 2, 8, 16, 32)  # [k, v, i, H, ch, w, j]
        # token t = 512H + 256v + 32ch + 16k + w ; element (i, j)
        o = d.transpose(3, 1, 4, 0, 5, 2, 6).reshape(T_CORE, 1024)
        outs.append(o)
    full = np.concatenate(outs, axis=0)                   # [8192, 1024]
    return full.reshape(x.shape[0], x.shape[1], 1024).astype(np.float32)


if __name__ == "__main__":
    rng = np.random.default_rng(0)
    x = rng.standard_normal((4, 2048, D), dtype=np.float32)
    w = (rng.standard_normal((H, D, 64), dtype=np.float32)
         * np.sqrt(2.0 / (D + 64))).astype(np.float32)
    o = kernel(x, w)
    print(o.shape, o.dtype)


# revision 4
# speedup vs baseline: 1.0728x; 1.0054x over previous
"""Trainium2 Bass kernel for nn_DynamLinear, v2.

Math (see reference.py): y = einsum('td,hdo->tho', x, W); a = y[...,:32],
b = y[...,32:]; S[t] = sum_h a[t,h,:] (x) b[t,h,:]  (16x32^T @ 16x32);
out = S / sqrt(mean_ij(S^2) + 16e-12)   (rms_norm is scale invariant).

Per-core plan (tokens sharded 1024/core, t = 512*H + 256*v + g):
  stage1: y^T = Wp^T @ x^T on TensorE (bf16), psum [128 cols, 2, 512 t] per
          (col-tile pair, half H); ACT/DVE/Pool copy -> y_sb[col, t] bf16.
  shuffle (through HBM): store y_sb -> ybuf[row = 512*sel+128*c+16*m+h, t]
          (4 quarter-stores x 2 sel, partition-uniform APs), then load
          ab[16*v+h, slot, g] <- ybuf with slot = 32*v+i for A (i = 8c+m)
          and 64+j for B.  Off-block A slots are pre-zeroed once from HBM.
          Load out-APs are emitted slot-major ([32 slot, 16 part, 256 g])
          so the cost model's per-queue charge reflects the actual 512B
          descriptor stream instead of a 16-partition-wide transfer.
  stage2: per group g one matmul lhsT = ab[0:32, 0:64, g] (block-diag
          A_t0/A_t1), rhs = ab[0:32, 64:96, g] -> S for 2 tokens at psum
          partitions [0:64); a second matmul at tile column 64 packs 2 more
          tokens -> full [128, 16, 32] psum chunks for full-width RMS.
  rms:    square (ACT), reduce_j (DVE 2x bf16), block-ones matmul reduces i
          and broadcasts (PE), Rsqrt (ACT), multiply (DVE/Pool), bf16 out.
"""

import sys
import functools
from contextlib import ExitStack

import numpy as np
import ml_dtypes

if "/opt/trn_rl_repo" not in sys.path:
    sys.path.insert(0, "/opt/trn_rl_repo")

import concourse.bass as bass
import concourse.bacc as bacc
import concourse.tile as tile
from concourse import mybir
from concourse.bass_utils import run_bass_kernel_spmd

N_CORES = 8
T_CORE = 1024          # tokens per core
D = 1024               # feat dim
H = 16                 # codebooks
EPS = 16e-12           # 16 * 1e-12 (scale-folded reference eps)

F32 = mybir.dt.float32
BF16 = mybir.dt.bfloat16
AF = mybir.ActivationFunctionType


def _kernel_body(tc, ctx, xt, wp, ind, zz, ybuf, out):
    nc = tc.nc

    singles = ctx.enter_context(tc.tile_pool(name="singles", bufs=1))
    py = ctx.enter_context(tc.tile_pool(name="py", bufs=3, space="PSUM"))
    ps2p = ctx.enter_context(tc.tile_pool(name="ps2", bufs=4, space="PSUM"))
    ps3p = ctx.enter_context(tc.tile_pool(name="ps3", bufs=1, space="PSUM"))
    ps3_all = ps3p.tile([128, 16, 16], F32)
    sqp = ctx.enter_context(tc.tile_pool(name="sqp", bufs=6))
    smalls = ctx.enter_context(tc.tile_pool(name="smalls", bufs=6))

    # ---- resident SBUF tensors ------------------------------------------
    wp_sb = singles.tile([128, 8, 8, 128], BF16)   # [dp, CT, dt, col]
    xt_sb = singles.tile([128, 8, 1024], BF16)     # [dp, dt, t]
    y_sb = singles.tile([128, 8, 1024], BF16)      # [colp, CT, t]
    # slots: A-H0 [0:64), A-H1 [64:128), B-H0 [128:160), B-H1 [160:192)
    ab = singles.tile([32, 192, 256], BF16)        # [16v+h, slot, g]
    ind_sb = singles.tile([128, 128], BF16)        # 4x (32x32 ones) blk diag
    zout = singles.tile([128, 2, 8, 512], BF16)    # [p, H, ch, (w j)]
    eps_sb = singles.tile([128, 1], F32)

    nc.sync.dma_start(out=xt_sb[:, 0:2, 0:512], in_=xt[:, 0:2, 0:512])
    nc.scalar.dma_start(out=wp_sb[:, 0:1], in_=wp[:, 0:1])
    nc.scalar.dma_start(out=wp_sb[:, 1:2], in_=wp[:, 1:2])
    nc.sync.dma_start(out=xt_sb[:, 2:5, 0:512], in_=xt[:, 2:5, 0:512])
    nc.sync.dma_start(out=xt_sb[:, 5:8, 0:512], in_=xt[:, 5:8, 0:512])
    nc.scalar.dma_start(out=wp_sb[:, 2:4], in_=wp[:, 2:4])
    nc.sync.dma_start(out=xt_sb[:, :, 512:1024], in_=xt[:, :, 512:1024])
    nc.scalar.dma_start(out=wp_sb[:, 4:8], in_=wp[:, 4:8])
    nc.gpsimd.dma_start(out=ind_sb[:], in_=ind[:])
    # zero the off-block A slots (both A buffers) from the zz input
    zz3 = zz[:].rearrange("p (s g) -> p s g", s=32)
    zeng = [nc.gpsimd, nc.gpsimd, nc.gpsimd, nc.sync,
            nc.gpsimd, nc.gpsimd, nc.sync, nc.gpsimd]
    zi = 0
    for Hh in range(2):
        a0 = 64 * Hh
        for (p0, s0) in ((0, 32), (16, 0)):
            for half in range(2):
                zeng[zi].dma_start(
                    out=ab[p0:p0 + 16, a0 + s0 + 16 * half:
                           a0 + s0 + 16 * half + 16, :],
                    in_=zz3[:, 16 * half:16 * half + 16, :])
                zi += 1
    nc.vector.memset(eps_sb[:], EPS)

    # views of ybuf [1024 rows = (sel, c, p=8h+m), 1024 t]
    yb_st = ybuf.rearrange("(sel c p) t -> sel p c t", sel=2, c=4)
    yb_a = ybuf.rearrange("(sel c h m) t -> sel c h m t", sel=2, c=4, h=16)
    yb_b = ybuf.rearrange("(sel c p) t -> sel c p t", sel=2, c=4)

    def _copy(dst, ps, e):
        if e == "act":
            nc.scalar.activation(dst, ps, AF.Copy)
        elif e == "dve":
            nc.vector.tensor_copy(dst, ps)
        else:
            nc.gpsimd.tensor_copy(dst, ps)

    def stage1_pair(Hh, pr, eng, split=False):
        t0 = 512 * Hh
        for u in range(2):
            CT = 2 * pr + u
            ps = py.tile([128, 512], F32)
            for d in range(8):
                nc.tensor.matmul(
                    ps[:],
                    lhsT=wp_sb[:, CT, d, :],
                    rhs=xt_sb[:, d, t0:t0 + 512],
                    start=(d == 0),
                    stop=(d == 7),
                )
            e = eng if u == 0 else eng2[eng]
            if split:
                _copy(y_sb[:, CT, t0:t0 + 256], ps[:, 0:256], "act")
                _copy(y_sb[:, CT, t0 + 256:t0 + 512], ps[:, 256:512], "dve")
            else:
                _copy(y_sb[:, CT, t0:t0 + 512], ps[:], e)

    def store_q(q, sel, eng, c0=0, ncc=4):
        eng.dma_start(
            out=yb_st[sel][:, c0:c0 + ncc, 256 * q:256 * q + 256],
            in_=y_sb[:, 4 * sel + c0:4 * sel + c0 + ncc,
                     256 * q:256 * q + 256],
        )

    def load_a(Hh, v, c, eng):
        # block-diag A piece: ab[16v+h, 64H+32v+8c+m, g] <- ybuf(A, c, h, m)
        q = 2 * Hh + v
        s0 = 64 * Hh + 32 * v + 8 * c
        eng.dma_start(
            out=ab[16 * v:16 * v + 16, s0:s0 + 8, :],
            in_=yb_a[0, c][:, :, 256 * q:256 * q + 256],
        )

    def load_b(Hh, c, eng):
        # stacked B piece: ab[16v+h (all 32), 128+32H+8c+m, g] <- ybuf(B, c)
        s0 = 128 + 32 * Hh + 8 * c
        eng.dma_start(
            out=ab[0:32, s0:s0 + 8, :],
            in_=yb_b[1, c][:, 512 * Hh:512 * Hh + 512].rearrange(
                "p (v g) -> v p g", v=2),
        )

    def stage2_chunk(Hh, ch, cp_eng, w0=0, nw=16):
        ps2 = ps2p.tile([128, nw, 32], F32)
        for w in range(nw):
            for k in range(2):
                g = 32 * ch + 16 * k + w0 + w
                nc.tensor.matmul(
                    ps2[64 * k:64 * k + 64, w, :],
                    lhsT=ab[0:32, 64 * Hh:64 * Hh + 64, g],
                    rhs=ab[0:32, 128 + 32 * Hh:160 + 32 * Hh, g],
                    start=True, stop=True,
                )
        # copy S to SBUF so square and the final multiply can run on Pool
        s_sb = sqp.tile([128, nw, 32], BF16)
        if cp_eng == "act":
            nc.scalar.activation(s_sb[:], ps2[:], AF.Copy)
        else:
            nc.vector.tensor_copy(s_sb[:], ps2[:])
        sq = sqp.tile([128, nw, 32], BF16)
        nc.gpsimd.tensor_mul(sq[:], s_sb[:], s_sb[:])
        part = smalls.tile([128, nw], BF16)
        with nc.allow_low_precision(reason="bf16 rms partial sums"):
            nc.vector.tensor_reduce(part[:], sq[:],
                                    axis=mybir.AxisListType.X,
                                    op=mybir.AluOpType.add)
        ps3 = ps3_all[:, 8 * Hh + ch, 0:nw]
        nc.tensor.matmul(ps3, lhsT=ind_sb[:], rhs=part[:],
                         start=True, stop=True)
        s_rms = smalls.tile([128, nw], F32)
        nc.scalar.activation(s_rms[:], ps3, AF.Sqrt,
                             bias=eps_sb[:], scale=1.0 / 1024.0)
        rstd = smalls.tile([128, nw], F32)
        nc.vector.reciprocal(rstd[:], s_rms[:])
        zv = zout[:, Hh, ch].rearrange("p (w j) -> p w j", w=16)[:, w0:w0 + nw]
        nc.gpsimd.tensor_mul(zv, s_sb[:],
                             rstd[:].unsqueeze(2).broadcast_to([128, nw, 32]))

    def store_out(Hh, c0, eng, n=4):
        eng.dma_start(out=out[:, Hh, c0:c0 + n, :],
                      in_=zout[:, Hh, c0:c0 + n, :])

    mul_engs_unused = None
    copy_engs = ["dve", "dve", "dve", "act", "dve", "dve", "dve", "dve"]
    eng2 = {"dve": "act", "act": "dve", "pool": "dve"}
    mul_engs = [nc.vector] * 16

    # ================= schedule =================
    # ---- half 0 stage 1 ----
    stage1_pair(0, 0, copy_engs[0])
    stage1_pair(0, 1, copy_engs[1])
    store_q(0, 0, nc.sync)
    store_q(1, 0, nc.sync)
    for c in range(4):
        load_a(0, 0, c, nc.sync if c % 2 == 0 else nc.scalar)
        load_a(0, 1, c, nc.scalar if c % 2 == 0 else nc.sync)
    stage1_pair(0, 2, copy_engs[2])
    store_q(0, 1, nc.sync, c0=0, ncc=2)
    store_q(1, 1, nc.scalar, c0=0, ncc=2)
    load_b(0, 0, nc.sync)
    load_b(0, 1, nc.gpsimd)
    stage1_pair(0, 3, copy_engs[3], split=True)
    store_q(0, 1, nc.sync, c0=2, ncc=2)
    store_q(1, 1, nc.scalar, c0=2, ncc=2)
    load_b(0, 2, nc.scalar)
    load_b(0, 3, nc.sync)

    # ---- half 1 stage 1, interleaved with half-0 stage 2 ----
    stage1_pair(1, 0, copy_engs[4])
    stage1_pair(1, 1, copy_engs[5])
    store_q(2, 0, nc.sync)
    store_q(3, 0, nc.sync)
    for c in range(4):
        load_a(1, 0, c, nc.sync if c % 2 == 0 else nc.gpsimd)
        load_a(1, 1, c, nc.gpsimd if c % 2 == 0 else nc.sync)
    for ch in range(4):
        stage2_chunk(0, ch, "act" if ch % 2 == 0 else "dve")
    stage1_pair(1, 2, copy_engs[6])
    store_q(2, 1, nc.sync, c0=0, ncc=2)
    store_q(3, 1, nc.scalar, c0=0, ncc=2)
    load_b(1, 0, nc.sync)
    load_b(1, 1, nc.gpsimd)
    stage1_pair(1, 3, copy_engs[7], split=True)
    store_q(2, 1, nc.sync, c0=2, ncc=2)
    store_q(3, 1, nc.scalar, c0=2, ncc=2)
    load_b(1, 2, nc.scalar)
    load_b(1, 3, nc.sync)
    for ch in range(4, 8):
        stage2_chunk(0, ch, "act" if ch % 2 == 0 else "dve")
    store_out(0, 0, nc.sync)
    store_out(0, 4, nc.sync)

    # ---- half 1 stage 2 ----
    h1_muls = [nc.vector] * 8
    for ch in range(8):
        stage2_chunk(1, ch, "act" if ch % 2 == 0 else "dve")
        if ch == 3:
            store_out(1, 0, nc.sync)
        if ch == 5:
            store_out(1, 4, nc.sync, n=2)
        if ch == 7:
            store_out(1, 6, nc.sync, n=2)


def build_program(trace_sim=False):
    nc = bacc.Bacc("TRN2", target_bir_lowering=False, debug=False)
    xt = nc.dram_tensor("xt", [128, 8, 1024], BF16, kind="ExternalInput").ap()
    wp = nc.dram_tensor("wp", [128, 8, 8, 128], BF16, kind="ExternalInput").ap()
    ind = nc.dram_tensor("ind", [128, 128], BF16, kind="ExternalInput").ap()
    zz = nc.dram_tensor("zz", [16, 8192], BF16, kind="ExternalInput").ap()
    ybuf = nc.dram_tensor("ybuf", [1024, 1024], BF16, kind="Internal").ap()
    out = nc.dram_tensor("out", [128, 2, 8, 512], BF16,
                         kind="ExternalOutput").ap()
    with tile.TileContext(nc, trace_sim=trace_sim) as tc:
        with ExitStack() as ctx:
            _kernel_body(tc, ctx, xt, wp, ind, zz, ybuf, out)
    if not trace_sim:
        nc.compile()
    return nc


@functools.lru_cache(maxsize=1)
def _built_program():
    return build_program(trace_sim=False)


def _host_prep(x, weight):
    xf = np.ascontiguousarray(x.reshape(-1, D))          # [8192, 1024]
    # Wp column order: col = 512*sel + 128*c + 8*h + m ; i = 8*c + m
    w = weight.transpose(1, 0, 2).reshape(D, H, 2, 4, 8)  # [d, h, sel, c, m]
    wp = w.transpose(0, 2, 3, 1, 4).reshape(D, 1024)      # [d, col]
    wp_sb = np.ascontiguousarray(
        wp.reshape(8, 128, 8, 128).transpose(1, 2, 0, 3)).astype(
            ml_dtypes.bfloat16)
    ind = np.kron(np.eye(4, dtype=np.float32),
                  np.ones((32, 32), dtype=np.float32)).astype(ml_dtypes.bfloat16)
    zz = np.zeros((16, 8192), dtype=ml_dtypes.bfloat16)
    xt_shards = []
    for c in range(N_CORES):
        xtc = xf[c * T_CORE:(c + 1) * T_CORE].T            # [d, t]
        xt_sb = np.ascontiguousarray(
            xtc.reshape(8, 128, 1024).transpose(1, 0, 2)).astype(
                ml_dtypes.bfloat16)
        xt_shards.append(xt_sb)
    return xt_shards, wp_sb, ind, zz


def kernel(x, weight, **_unused):
    x = np.asarray(x, dtype=np.float32)
    weight = np.asarray(weight, dtype=np.float32)
    xt_shards, wp_sb, ind, zz = _host_prep(x, weight)
    nc = _built_program()
    in_maps = [{"xt": xt_shards[c], "wp": wp_sb, "ind": ind, "zz": zz}
               for c in range(N_CORES)]
    res = run_bass_kernel_spmd(nc, in_maps, list(range(N_CORES)))
    outs = []
    for c in range(N_CORES):
        d = np.asarray(res.results[c]["out"]).astype(np.float32)
        d = d.reshape(2, 2, 32, 2, 8, 16, 32)  # [k, v, i, H, ch, w, j]
        # token t = 512H + 256v + 32ch + 16k + w ; element (i, j)
        o = d.transpose(3, 1, 4, 0, 5, 2, 6).reshape(T_CORE, 1024)
        outs.append(o)
    full = np.concatenate(outs, axis=0)                   # [8192, 1024]
    return full.reshape(x.shape[0], x.shape[1], 1024).astype(np.float32)


if __name__ == "__main__":
    rng = np.random.default_rng(0)
    x = rng.standard_normal((4, 2048, D), dtype=np.float32)
    w = (rng.standard_normal((H, D, 64), dtype=np.float32)
         * np.sqrt(2.0 / (D + 64))).astype(np.float32)
    o = kernel(x, w)
    print(o.shape, o.dtype)


# revision 5
# speedup vs baseline: 1.0796x; 1.0063x over previous
"""Trainium2 Bass kernel for nn_DynamLinear, v2.

Math (see reference.py): y = einsum('td,hdo->tho', x, W); a = y[...,:32],
b = y[...,32:]; S[t] = sum_h a[t,h,:] (x) b[t,h,:]  (16x32^T @ 16x32);
out = S / sqrt(mean_ij(S^2) + 16e-12)   (rms_norm is scale invariant).

Per-core plan (tokens sharded 1024/core, t = 512*H + 256*v + g):
  stage1: y^T = Wp^T @ x^T on TensorE (bf16), psum [128 cols, 2, 512 t] per
          (col-tile pair, half H); ACT/DVE/Pool copy -> y_sb[col, t] bf16.
  shuffle (through HBM): store y_sb -> ybuf[row = 512*sel+128*c+16*m+h, t]
          (4 quarter-stores x 2 sel, partition-uniform APs), then load
          ab[16*v+h, slot, g] <- ybuf with slot = 32*v+i for A (i = 8c+m)
          and 64+j for B.  Off-block A slots are pre-zeroed once from HBM.
          Load out-APs are emitted slot-major ([32 slot, 16 part, 256 g])
          so the cost model's per-queue charge reflects the actual 512B
          descriptor stream instead of a 16-partition-wide transfer.
  stage2: per group g one matmul lhsT = ab[0:32, 0:64, g] (block-diag
          A_t0/A_t1), rhs = ab[0:32, 64:96, g] -> S for 2 tokens at psum
          partitions [0:64); a second matmul at tile column 64 packs 2 more
          tokens -> full [128, 16, 32] psum chunks for full-width RMS.
  rms:    square (ACT), reduce_j (DVE 2x bf16), block-ones matmul reduces i
          and broadcasts (PE), Rsqrt (ACT), multiply (DVE/Pool), bf16 out.
"""

import sys
import functools
from contextlib import ExitStack

import numpy as np
import ml_dtypes

if "/opt/trn_rl_repo" not in sys.path:
    sys.path.insert(0, "/opt/trn_rl_repo")

import concourse.bass as bass
import concourse.bacc as bacc
import concourse.tile as tile
from concourse import mybir
from concourse.bass_utils import run_bass_kernel_spmd

N_CORES = 8
T_CORE = 1024          # tokens per core
D = 1024               # feat dim
H = 16                 # codebooks
EPS = 16e-12           # 16 * 1e-12 (scale-folded reference eps)

F32 = mybir.dt.float32
BF16 = mybir.dt.bfloat16
AF = mybir.ActivationFunctionType


def _kernel_body(tc, ctx, xt, wp, ind, zz, ybuf, out):
    nc = tc.nc

    singles = ctx.enter_context(tc.tile_pool(name="singles", bufs=1))
    py = ctx.enter_context(tc.tile_pool(name="py", bufs=3, space="PSUM"))
    ps2p = ctx.enter_context(tc.tile_pool(name="ps2", bufs=4, space="PSUM"))
    ps3p = ctx.enter_context(tc.tile_pool(name="ps3", bufs=1, space="PSUM"))
    ps3_all = ps3p.tile([128, 16, 16], F32)
    sqp = ctx.enter_context(tc.tile_pool(name="sqp", bufs=6))
    smalls = ctx.enter_context(tc.tile_pool(name="smalls", bufs=6))

    # ---- resident SBUF tensors ------------------------------------------
    wp_sb = singles.tile([128, 8, 8, 128], BF16)   # [dp, CT, dt, col]
    xt_sb = singles.tile([128, 8, 1024], BF16)     # [dp, dt, t]
    y_sb = singles.tile([128, 8, 1024], BF16)      # [colp, CT, t]
    # slots: A-H0 [0:64), A-H1 [64:128), B-H0 [128:160), B-H1 [160:192)
    ab = singles.tile([32, 192, 256], BF16)        # [16v+h, slot, g]
    ind_sb = singles.tile([128, 128], BF16)        # 4x (32x32 ones) blk diag
    zout = singles.tile([128, 2, 8, 512], BF16)    # [p, H, ch, (w j)]
    eps_sb = singles.tile([128, 1], F32)

    nc.sync.dma_start(out=xt_sb[:, 0:2, 0:512], in_=xt[:, 0:2, 0:512])
    nc.scalar.dma_start(out=wp_sb[:, 0:1], in_=wp[:, 0:1])
    nc.scalar.dma_start(out=wp_sb[:, 1:2], in_=wp[:, 1:2])
    nc.sync.dma_start(out=xt_sb[:, 2:5, 0:512], in_=xt[:, 2:5, 0:512])
    nc.sync.dma_start(out=xt_sb[:, 5:8, 0:512], in_=xt[:, 5:8, 0:512])
    nc.scalar.dma_start(out=wp_sb[:, 2:4], in_=wp[:, 2:4])
    nc.sync.dma_start(out=xt_sb[:, :, 512:1024], in_=xt[:, :, 512:1024])
    nc.scalar.dma_start(out=wp_sb[:, 4:8], in_=wp[:, 4:8])
    nc.gpsimd.dma_start(out=ind_sb[:], in_=ind[:])
    # zero the off-block A slots (both A buffers) from the zz input
    zz3 = zz[:].rearrange("p (s g) -> p s g", s=32)
    zeng = [nc.gpsimd, nc.gpsimd, nc.gpsimd, nc.sync,
            nc.gpsimd, nc.gpsimd, nc.sync, nc.gpsimd]
    zi = 0
    for Hh in range(2):
        a0 = 64 * Hh
        for (p0, s0) in ((0, 32), (16, 0)):
            for half in range(2):
                zeng[zi].dma_start(
                    out=ab[p0:p0 + 16, a0 + s0 + 16 * half:
                           a0 + s0 + 16 * half + 16, :],
                    in_=zz3[:, 16 * half:16 * half + 16, :])
                zi += 1
    nc.vector.memset(eps_sb[:], EPS)

    # views of ybuf [1024 rows = (sel, c, p=8h+m), 1024 t]
    yb_st = ybuf.rearrange("(sel c p) t -> sel p c t", sel=2, c=4)
    yb_a = ybuf.rearrange("(sel c h m) t -> sel c h m t", sel=2, c=4, h=16)
    yb_b = ybuf.rearrange("(sel c p) t -> sel c p t", sel=2, c=4)

    def _copy(dst, ps, e):
        if e == "act":
            nc.scalar.activation(dst, ps, AF.Copy)
        elif e == "dve":
            nc.vector.tensor_copy(dst, ps)
        else:
            nc.gpsimd.tensor_copy(dst, ps)

    def stage1_pair(Hh, pr, eng, split=False):
        t0 = 512 * Hh
        for u in range(2):
            CT = 2 * pr + u
            ps = py.tile([128, 512], F32)
            for d in range(8):
                nc.tensor.matmul(
                    ps[:],
                    lhsT=wp_sb[:, CT, d, :],
                    rhs=xt_sb[:, d, t0:t0 + 512],
                    start=(d == 0),
                    stop=(d == 7),
                )
            e = eng if u == 0 else eng2[eng]
            if split:
                _copy(y_sb[:, CT, t0:t0 + 256], ps[:, 0:256], "act")
                _copy(y_sb[:, CT, t0 + 256:t0 + 512], ps[:, 256:512], "dve")
            else:
                _copy(y_sb[:, CT, t0:t0 + 512], ps[:], e)

    def store_q(q, sel, eng, c0=0, ncc=4):
        eng.dma_start(
            out=yb_st[sel][:, c0:c0 + ncc, 256 * q:256 * q + 256],
            in_=y_sb[:, 4 * sel + c0:4 * sel + c0 + ncc,
                     256 * q:256 * q + 256],
        )

    def load_a(Hh, v, c, eng):
        # block-diag A piece: ab[16v+h, 64H+32v+8c+m, g] <- ybuf(A, c, h, m)
        q = 2 * Hh + v
        s0 = 64 * Hh + 32 * v + 8 * c
        eng.dma_start(
            out=ab[16 * v:16 * v + 16, s0:s0 + 8, :],
            in_=yb_a[0, c][:, :, 256 * q:256 * q + 256],
        )

    def load_b(Hh, c, eng):
        # stacked B piece: ab[16v+h (all 32), 128+32H+8c+m, g] <- ybuf(B, c)
        s0 = 128 + 32 * Hh + 8 * c
        eng.dma_start(
            out=ab[0:32, s0:s0 + 8, :],
            in_=yb_b[1, c][:, 512 * Hh:512 * Hh + 512].rearrange(
                "p (v g) -> v p g", v=2),
        )

    def stage2_chunk(Hh, ch, cp_eng, w0=0, nw=16):
        ps2 = ps2p.tile([128, nw, 32], F32)
        for w in range(nw):
            for k in range(2):
                g = 32 * ch + 16 * k + w0 + w
                nc.tensor.matmul(
                    ps2[64 * k:64 * k + 64, w, :],
                    lhsT=ab[0:32, 64 * Hh:64 * Hh + 64, g],
                    rhs=ab[0:32, 128 + 32 * Hh:160 + 32 * Hh, g],
                    start=True, stop=True,
                )
        # copy S to SBUF so square and the final multiply can run on Pool
        s_sb = sqp.tile([128, nw, 32], BF16)
        if cp_eng == "act":
            nc.scalar.activation(s_sb[:], ps2[:], AF.Copy)
        else:
            nc.vector.tensor_copy(s_sb[:], ps2[:])
        sq = sqp.tile([128, nw, 32], BF16)
        nc.gpsimd.tensor_mul(sq[:], s_sb[:], s_sb[:])
        part = smalls.tile([128, nw], BF16)
        with nc.allow_low_precision(reason="bf16 rms partial sums"):
            nc.vector.tensor_reduce(part[:], sq[:],
                                    axis=mybir.AxisListType.X,
                                    op=mybir.AluOpType.add)
        ps3 = ps3_all[:, 8 * Hh + ch, 0:nw]
        nc.tensor.matmul(ps3, lhsT=ind_sb[:], rhs=part[:],
                         start=True, stop=True)
        s_rms = smalls.tile([128, nw], F32)
        nc.scalar.activation(s_rms[:], ps3, AF.Sqrt,
                             bias=eps_sb[:], scale=1.0 / 1024.0)
        rstd = smalls.tile([128, nw], F32)
        nc.vector.reciprocal(rstd[:], s_rms[:])
        zv = zout[:, Hh, ch].rearrange("p (w j) -> p w j", w=16)[:, w0:w0 + nw]
        nc.gpsimd.tensor_mul(zv, s_sb[:],
                             rstd[:].unsqueeze(2).broadcast_to([128, nw, 32]))

    def store_out(Hh, c0, eng, n=4):
        eng.dma_start(out=out[:, Hh, c0:c0 + n, :],
                      in_=zout[:, Hh, c0:c0 + n, :])

    mul_engs_unused = None
    copy_engs = ["dve", "dve", "dve", "act", "dve", "dve", "dve", "dve"]
    eng2 = {"dve": "act", "act": "dve", "pool": "dve"}
    mul_engs = [nc.vector] * 16

    # ================= schedule =================
    # ---- half 0 stage 1 ----
    stage1_pair(0, 0, copy_engs[0])
    stage1_pair(0, 1, copy_engs[1])
    store_q(0, 0, nc.sync)
    store_q(1, 0, nc.sync)
    for c in range(4):
        load_a(0, 0, c, nc.sync if c % 2 == 0 else nc.scalar)
        load_a(0, 1, c, nc.scalar if c % 2 == 0 else nc.sync)
    stage1_pair(0, 2, copy_engs[2])
    store_q(0, 1, nc.sync, c0=0, ncc=2)
    store_q(1, 1, nc.scalar, c0=0, ncc=2)
    load_b(0, 0, nc.sync)
    load_b(0, 1, nc.gpsimd)
    stage1_pair(0, 3, copy_engs[3], split=True)
    store_q(0, 1, nc.sync, c0=2, ncc=2)
    store_q(1, 1, nc.scalar, c0=2, ncc=2)
    load_b(0, 2, nc.scalar)
    load_b(0, 3, nc.sync)

    # ---- half 1 stage 1, interleaved with half-0 stage 2 ----
    stage1_pair(1, 0, copy_engs[4])
    stage1_pair(1, 1, copy_engs[5])
    store_q(2, 0, nc.sync)
    store_q(3, 0, nc.sync)
    for c in range(4):
        load_a(1, 0, c, nc.sync if c % 2 == 0 else nc.gpsimd)
        load_a(1, 1, c, nc.gpsimd if c % 2 == 0 else nc.sync)
    for ch in range(4):
        stage2_chunk(0, ch, "act" if ch % 2 == 0 else "dve")
    stage1_pair(1, 2, copy_engs[6])
    store_q(2, 1, nc.sync, c0=0, ncc=2)
    store_q(3, 1, nc.scalar, c0=0, ncc=2)
    load_b(1, 0, nc.sync)
    load_b(1, 1, nc.gpsimd)
    stage1_pair(1, 3, copy_engs[7], split=True)
    store_q(2, 1, nc.sync, c0=2, ncc=2)
    store_q(3, 1, nc.scalar, c0=2, ncc=2)
    load_b(1, 2, nc.scalar)
    load_b(1, 3, nc.sync)
    for ch in range(4, 8):
        stage2_chunk(0, ch, "act" if ch % 2 == 0 else "dve")
    store_out(0, 0, nc.sync)
    store_out(0, 4, nc.sync)

    # ---- half 1 stage 2 ----
    h1_muls = [nc.vector] * 8
    for ch in range(8):
        stage2_chunk(1, ch, "act" if ch % 2 == 0 else "dve")
        if ch == 1:
            store_out(1, 0, nc.sync, n=2)
        if ch == 3:
            store_out(1, 2, nc.sync, n=2)
        if ch == 4:
            store_out(1, 4, nc.sync, n=1)
        if ch == 5:
            store_out(1, 5, nc.sync, n=1)
        if ch == 6:
            store_out(1, 6, nc.sync, n=1)
        if ch == 7:
            store_out(1, 7, nc.sync, n=1)


def build_program(trace_sim=False):
    nc = bacc.Bacc("TRN2", target_bir_lowering=False, debug=False)
    xt = nc.dram_tensor("xt", [128, 8, 1024], BF16, kind="ExternalInput").ap()
    wp = nc.dram_tensor("wp", [128, 8, 8, 128], BF16, kind="ExternalInput").ap()
    ind = nc.dram_tensor("ind", [128, 128], BF16, kind="ExternalInput").ap()
    zz = nc.dram_tensor("zz", [16, 8192], BF16, kind="ExternalInput").ap()
    ybuf = nc.dram_tensor("ybuf", [1024, 1024], BF16, kind="Internal").ap()
    out = nc.dram_tensor("out", [128, 2, 8, 512], BF16,
                         kind="ExternalOutput").ap()
    with tile.TileContext(nc, trace_sim=trace_sim) as tc:
        with ExitStack() as ctx:
            _kernel_body(tc, ctx, xt, wp, ind, zz, ybuf, out)
    if not trace_sim:
        nc.compile()
    return nc


@functools.lru_cache(maxsize=1)
def _built_program():
    return build_program(trace_sim=False)


def _host_prep(x, weight):
    xf = np.ascontiguousarray(x.reshape(-1, D))          # [8192, 1024]
    # Wp column order: col = 512*sel + 128*c + 8*h + m ; i = 8*c + m
    w = weight.transpose(1, 0, 2).reshape(D, H, 2, 4, 8)  # [d, h, sel, c, m]
    wp = w.transpose(0, 2, 3, 1, 4).reshape(D, 1024)      # [d, col]
    wp_sb = np.ascontiguousarray(
        wp.reshape(8, 128, 8, 128).transpose(1, 2, 0, 3)).astype(
            ml_dtypes.bfloat16)
    ind = np.kron(np.eye(4, dtype=np.float32),
                  np.ones((32, 32), dtype=np.float32)).astype(ml_dtypes.bfloat16)
    zz = np.zeros((16, 8192), dtype=ml_dtypes.bfloat16)
    xt_shards = []
    for c in range(N_CORES):
        xtc = xf[c * T_CORE:(c + 1) * T_CORE].T            # [d, t]
        xt_sb = np.ascontiguousarray(
            xtc.reshape(8, 128, 1024).transpose(1, 0, 2)).astype(
                ml_dtypes.bfloat16)
        xt_shards.append(xt_sb)
    return xt_shards, wp_sb, ind, zz


def kernel(x, weight, **_unused):
    x = np.asarray(x, dtype=np.float32)
    weight = np.asarray(weight, dtype=np.float32)
    xt_shards, wp_sb, ind, zz = _host_prep(x, weight)
    nc = _built_program()
    in_maps = [{"xt": xt_shards[c], "wp": wp_sb, "ind": ind, "zz": zz}
               for c in range(N_CORES)]
    res = run_bass_kernel_spmd(nc, in_maps, list(range(N_CORES)))
    outs = []
    for c in range(N_CORES):
        d = np.asarray(res.results[c]["out"]).astype(np.float32)
        d = d.reshape(2, 2, 32, 2, 8, 16, 32)  # [k, v, i, H, ch, w, j]
        # token t = 512H + 256v + 32ch + 16k + w ; element (i, j)
        o = d.transpose(3, 1, 4, 0, 5, 2, 6).reshape(T_CORE, 1024)
        outs.append(o)
    full = np.concatenate(outs, axis=0)                   # [8192, 1024]
    return full.reshape(x.shape[0], x.shape[1], 1024).astype(np.float32)


if __name__ == "__main__":
    rng = np.random.default_rng(0)
    x = rng.standard_normal((4, 2048, D), dtype=np.float32)
    w = (rng.standard_normal((H, D, 64), dtype=np.float32)
         * np.sqrt(2.0 / (D + 64))).astype(np.float32)
    o = kernel(x, w)
    print(o.shape, o.dtype)


# revision 6
# speedup vs baseline: 1.1042x; 1.0228x over previous
"""Trainium2 Bass kernel for nn_DynamLinear, v2.

Math (see reference.py): y = einsum('td,hdo->tho', x, W); a = y[...,:32],
b = y[...,32:]; S[t] = sum_h a[t,h,:] (x) b[t,h,:]  (16x32^T @ 16x32);
out = S / sqrt(mean_ij(S^2) + 16e-12)   (rms_norm is scale invariant).

Per-core plan (tokens sharded 1024/core, t = 512*H + 256*v + g):
  stage1: y^T = Wp^T @ x^T on TensorE (bf16), psum [128 cols, 2, 512 t] per
          (col-tile pair, half H); ACT/DVE/Pool copy -> y_sb[col, t] bf16.
  shuffle (through HBM): store y_sb -> ybuf[row = 512*sel+128*c+16*m+h, t]
          (4 quarter-stores x 2 sel, partition-uniform APs), then load
          ab[16*v+h, slot, g] <- ybuf with slot = 32*v+i for A (i = 8c+m)
          and 64+j for B.  Off-block A slots are pre-zeroed once from HBM.
          Load out-APs are emitted slot-major ([32 slot, 16 part, 256 g])
          so the cost model's per-queue charge reflects the actual 512B
          descriptor stream instead of a 16-partition-wide transfer.
  stage2: per group g one matmul lhsT = ab[0:32, 0:64, g] (block-diag
          A_t0/A_t1), rhs = ab[0:32, 64:96, g] -> S for 2 tokens at psum
          partitions [0:64); a second matmul at tile column 64 packs 2 more
          tokens -> full [128, 16, 32] psum chunks for full-width RMS.
  rms:    square (ACT), reduce_j (DVE 2x bf16), block-ones matmul reduces i
          and broadcasts (PE), Rsqrt (ACT), multiply (DVE/Pool), bf16 out.
"""

import sys
import functools
from contextlib import ExitStack

import numpy as np
import ml_dtypes

if "/opt/trn_rl_repo" not in sys.path:
    sys.path.insert(0, "/opt/trn_rl_repo")

import concourse.bass as bass
import concourse.bacc as bacc
import concourse.tile as tile
from concourse import mybir
from concourse.bass_utils import run_bass_kernel_spmd

N_CORES = 8
T_CORE = 1024          # tokens per core
D = 1024               # feat dim
H = 16                 # codebooks
EPS = 16e-12           # 16 * 1e-12 (scale-folded reference eps)

F32 = mybir.dt.float32
BF16 = mybir.dt.bfloat16
AF = mybir.ActivationFunctionType


def _kernel_body(tc, ctx, xt, wp, ind, zz, ybuf, out):
    nc = tc.nc

    singles = ctx.enter_context(tc.tile_pool(name="singles", bufs=1))
    py = ctx.enter_context(tc.tile_pool(name="py", bufs=4, space="PSUM"))
    ps2p = ctx.enter_context(tc.tile_pool(name="ps2", bufs=3, space="PSUM"))
    ps3p = ctx.enter_context(tc.tile_pool(name="ps3", bufs=1, space="PSUM"))
    ps3_all = ps3p.tile([128, 16, 16], F32)
    sqp = ctx.enter_context(tc.tile_pool(name="sqp", bufs=6))
    smalls = ctx.enter_context(tc.tile_pool(name="smalls", bufs=6))

    # ---- resident SBUF tensors ------------------------------------------
    wp_sb = singles.tile([128, 8, 8, 128], BF16)   # [dp, CT, dt, col]
    xt_sb = singles.tile([128, 8, 1024], BF16)     # [dp, dt, t]
    y_sb = singles.tile([128, 8, 1024], BF16)      # [colp, CT, t]
    # slots: A-H0 [0:64), A-H1 [64:128), B-H0 [128:160), B-H1 [160:192)
    ab = singles.tile([32, 192, 256], BF16)        # [16v+h, slot, g]
    ind_sb = singles.tile([128, 128], BF16)        # 4x (32x32 ones) blk diag
    zout = singles.tile([128, 2, 8, 512], BF16)    # [p, H, ch, (w j)]
    eps_sb = singles.tile([128, 1], F32)

    nc.sync.dma_start(out=xt_sb[:, 0:2, 0:512], in_=xt[:, 0:2, 0:512])
    nc.scalar.dma_start(out=wp_sb[:, 0:1], in_=wp[:, 0:1])
    nc.scalar.dma_start(out=wp_sb[:, 1:2], in_=wp[:, 1:2])
    nc.sync.dma_start(out=xt_sb[:, 2:5, 0:512], in_=xt[:, 2:5, 0:512])
    nc.sync.dma_start(out=xt_sb[:, 5:8, 0:512], in_=xt[:, 5:8, 0:512])
    nc.scalar.dma_start(out=wp_sb[:, 2:4], in_=wp[:, 2:4])
    nc.sync.dma_start(out=xt_sb[:, :, 512:1024], in_=xt[:, :, 512:1024])
    nc.scalar.dma_start(out=wp_sb[:, 4:8], in_=wp[:, 4:8])
    nc.gpsimd.dma_start(out=ind_sb[:], in_=ind[:])
    # zero the off-block A slots (both A buffers) from the zz input
    zz3 = zz[:].rearrange("p (s g) -> p s g", s=32)
    zeng = [nc.gpsimd, nc.gpsimd, nc.gpsimd, nc.sync,
            nc.gpsimd, nc.gpsimd, nc.sync, nc.gpsimd]
    zi = 0
    for Hh in range(2):
        a0 = 64 * Hh
        for (p0, s0) in ((0, 32), (16, 0)):
            for half in range(2):
                zeng[zi].dma_start(
                    out=ab[p0:p0 + 16, a0 + s0 + 16 * half:
                           a0 + s0 + 16 * half + 16, :],
                    in_=zz3[:, 16 * half:16 * half + 16, :])
                zi += 1
    nc.vector.memset(eps_sb[:], EPS)

    # views of ybuf [1024 rows = (sel, c, p=8h+m), 1024 t]
    yb_st = ybuf.rearrange("(sel c p) t -> sel p c t", sel=2, c=4)
    yb_a = ybuf.rearrange("(sel c h m) t -> sel c h m t", sel=2, c=4, h=16)
    yb_b = ybuf.rearrange("(sel c p) t -> sel c p t", sel=2, c=4)

    def _copy(dst, ps, e):
        if e == "act":
            nc.scalar.activation(dst, ps, AF.Copy)
        elif e == "dve":
            nc.vector.tensor_copy(dst, ps)
        else:
            nc.gpsimd.tensor_copy(dst, ps)

    def stage1_pair(Hh, pr, eng, split=False):
        t0 = 512 * Hh
        for u in range(2):
            CT = 2 * pr + u
            ps = py.tile([128, 512], F32)
            for d in range(8):
                nc.tensor.matmul(
                    ps[:],
                    lhsT=wp_sb[:, CT, d, :],
                    rhs=xt_sb[:, d, t0:t0 + 512],
                    start=(d == 0),
                    stop=(d == 7),
                )
            e = eng if u == 0 else eng2[eng]
            if split:
                _copy(y_sb[:, CT, t0:t0 + 256], ps[:, 0:256], "act")
                _copy(y_sb[:, CT, t0 + 256:t0 + 512], ps[:, 256:512], "dve")
            else:
                _copy(y_sb[:, CT, t0:t0 + 512], ps[:], e)

    def store_q(q, sel, eng, c0=0, ncc=4):
        eng.dma_start(
            out=yb_st[sel][:, c0:c0 + ncc, 256 * q:256 * q + 256],
            in_=y_sb[:, 4 * sel + c0:4 * sel + c0 + ncc,
                     256 * q:256 * q + 256],
        )

    def load_a(Hh, v, c, eng):
        # block-diag A piece: ab[16v+h, 64H+32v+8c+m, g] <- ybuf(A, c, h, m)
        q = 2 * Hh + v
        s0 = 64 * Hh + 32 * v + 8 * c
        eng.dma_start(
            out=ab[16 * v:16 * v + 16, s0:s0 + 8, :],
            in_=yb_a[0, c][:, :, 256 * q:256 * q + 256],
        )

    def load_b(Hh, c, eng):
        # stacked B piece: ab[16v+h (all 32), 128+32H+8c+m, g] <- ybuf(B, c)
        s0 = 128 + 32 * Hh + 8 * c
        eng.dma_start(
            out=ab[0:32, s0:s0 + 8, :],
            in_=yb_b[1, c][:, 512 * Hh:512 * Hh + 512].rearrange(
                "p (v g) -> v p g", v=2),
        )

    def stage2_chunk(Hh, ch, cp_eng, w0=0, nw=16):
        ps2 = ps2p.tile([128, nw, 32], F32)
        for w in range(nw):
            for k in range(2):
                g = 32 * ch + 16 * k + w0 + w
                nc.tensor.matmul(
                    ps2[64 * k:64 * k + 64, w, :],
                    lhsT=ab[0:32, 64 * Hh:64 * Hh + 64, g],
                    rhs=ab[0:32, 128 + 32 * Hh:160 + 32 * Hh, g],
                    start=True, stop=True,
                )
        # copy S to SBUF so square and the final multiply can run on Pool
        s_sb = sqp.tile([128, nw, 32], BF16)
        if cp_eng == "act":
            nc.scalar.activation(s_sb[:], ps2[:], AF.Copy)
        else:
            nc.vector.tensor_copy(s_sb[:], ps2[:])
        sq = sqp.tile([128, nw, 32], BF16)
        nc.gpsimd.tensor_mul(sq[:], s_sb[:], s_sb[:])
        part = smalls.tile([128, nw], BF16)
        with nc.allow_low_precision(reason="bf16 rms partial sums"):
            nc.vector.tensor_reduce(part[:], sq[:],
                                    axis=mybir.AxisListType.X,
                                    op=mybir.AluOpType.add)
        ps3 = ps3_all[:, 8 * Hh + ch, 0:nw]
        nc.tensor.matmul(ps3, lhsT=ind_sb[:], rhs=part[:],
                         start=True, stop=True)
        s_rms = smalls.tile([128, nw], F32)
        nc.scalar.activation(s_rms[:], ps3, AF.Sqrt,
                             bias=eps_sb[:], scale=1.0 / 1024.0)
        rstd = smalls.tile([128, nw], F32)
        nc.vector.reciprocal(rstd[:], s_rms[:])
        zv = zout[:, Hh, ch].rearrange("p (w j) -> p w j", w=16)[:, w0:w0 + nw]
        nc.gpsimd.tensor_mul(zv, s_sb[:],
                             rstd[:].unsqueeze(2).broadcast_to([128, nw, 32]))

    def store_out(Hh, c0, eng, n=4):
        eng.dma_start(out=out[:, Hh, c0:c0 + n, :],
                      in_=zout[:, Hh, c0:c0 + n, :])

    mul_engs_unused = None
    copy_engs = ["dve", "dve", "dve", "act", "dve", "dve", "dve", "dve"]
    eng2 = {"dve": "act", "act": "dve", "pool": "dve"}
    mul_engs = [nc.vector] * 16

    # ================= schedule =================
    # ---- half 0 stage 1 ----
    stage1_pair(0, 0, copy_engs[0])
    stage1_pair(0, 1, copy_engs[1])
    store_q(0, 0, nc.sync)
    store_q(1, 0, nc.sync)
    for c in range(4):
        load_a(0, 0, c, nc.sync if c % 2 == 0 else nc.scalar)
        load_a(0, 1, c, nc.scalar if c % 2 == 0 else nc.sync)
    stage1_pair(0, 2, copy_engs[2])
    store_q(0, 1, nc.sync, c0=0, ncc=2)
    store_q(1, 1, nc.scalar, c0=0, ncc=2)
    load_b(0, 0, nc.sync)
    load_b(0, 1, nc.gpsimd)
    stage1_pair(0, 3, copy_engs[3], split=True)
    store_q(0, 1, nc.sync, c0=2, ncc=2)
    store_q(1, 1, nc.scalar, c0=2, ncc=2)
    load_b(0, 2, nc.scalar)
    load_b(0, 3, nc.sync)

    # ---- half 1 stage 1, interleaved with half-0 stage 2 ----
    stage1_pair(1, 0, copy_engs[4])
    stage1_pair(1, 1, copy_engs[5])
    store_q(2, 0, nc.sync)
    store_q(3, 0, nc.sync)
    for c in range(4):
        load_a(1, 0, c, nc.sync if c % 2 == 0 else nc.gpsimd)
        load_a(1, 1, c, nc.gpsimd if c % 2 == 0 else nc.sync)
    for ch in range(4):
        stage2_chunk(0, ch, "act" if ch % 2 == 0 else "dve")
    stage1_pair(1, 2, copy_engs[6])
    store_q(2, 1, nc.sync, c0=0, ncc=2)
    store_q(3, 1, nc.scalar, c0=0, ncc=2)
    load_b(1, 0, nc.sync)
    load_b(1, 1, nc.gpsimd)
    stage1_pair(1, 3, copy_engs[7], split=True)
    store_q(2, 1, nc.sync, c0=2, ncc=2)
    store_q(3, 1, nc.scalar, c0=2, ncc=2)
    load_b(1, 2, nc.scalar)
    load_b(1, 3, nc.sync)
    for ch in range(4, 8):
        stage2_chunk(0, ch, "act" if ch % 2 == 0 else "dve")
    store_out(0, 0, nc.sync)
    store_out(0, 4, nc.sync)

    # ---- half 1 stage 2 ----
    h1_muls = [nc.vector] * 8
    for ch in range(8):
        stage2_chunk(1, ch, "act" if ch % 2 == 0 else "dve")
        if ch == 1:
            store_out(1, 0, nc.sync, n=2)
        if ch == 3:
            store_out(1, 2, nc.sync, n=2)
        if ch == 4:
            store_out(1, 4, nc.sync, n=1)
        if ch == 5:
            store_out(1, 5, nc.sync, n=1)
        if ch == 6:
            store_out(1, 6, nc.sync, n=1)
        if ch == 7:
            store_out(1, 7, nc.sync, n=1)


def build_program(trace_sim=False):
    nc = bacc.Bacc("TRN2", target_bir_lowering=False, debug=False)
    xt = nc.dram_tensor("xt", [128, 8, 1024], BF16, kind="ExternalInput").ap()
    wp = nc.dram_tensor("wp", [128, 8, 8, 128], BF16, kind="ExternalInput").ap()
    ind = nc.dram_tensor("ind", [128, 128], BF16, kind="ExternalInput").ap()
    zz = nc.dram_tensor("zz", [16, 8192], BF16, kind="ExternalInput").ap()
    ybuf = nc.dram_tensor("ybuf", [1024, 1024], BF16, kind="Internal").ap()
    out = nc.dram_tensor("out", [128, 2, 8, 512], BF16,
                         kind="ExternalOutput").ap()
    with tile.TileContext(nc, trace_sim=trace_sim) as tc:
        with ExitStack() as ctx:
            _kernel_body(tc, ctx, xt, wp, ind, zz, ybuf, out)
    if not trace_sim:
        nc.compile()
    return nc


@functools.lru_cache(maxsize=1)
def _built_program():
    return build_program(trace_sim=False)


def _host_prep(x, weight):
    xf = np.ascontiguousarray(x.reshape(-1, D))          # [8192, 1024]
    # Wp column order: col = 512*sel + 128*c + 8*h + m ; i = 8*c + m
    w = weight.transpose(1, 0, 2).reshape(D, H, 2, 4, 8)  # [d, h, sel, c, m]
    wp = w.transpose(0, 2, 3, 1, 4).reshape(D, 1024)      # [d, col]
    wp_sb = np.ascontiguousarray(
        wp.reshape(8, 128, 8, 128).transpose(1, 2, 0, 3)).astype(
            ml_dtypes.bfloat16)
    ind = np.kron(np.eye(4, dtype=np.float32),
                  np.ones((32, 32), dtype=np.float32)).astype(ml_dtypes.bfloat16)
    zz = np.zeros((16, 8192), dtype=ml_dtypes.bfloat16)
    xt_shards = []
    for c in range(N_CORES):
        xtc = xf[c * T_CORE:(c + 1) * T_CORE].T            # [d, t]
        xt_sb = np.ascontiguousarray(
            xtc.reshape(8, 128, 1024).transpose(1, 0, 2)).astype(
                ml_dtypes.bfloat16)
        xt_shards.append(xt_sb)
    return xt_shards, wp_sb, ind, zz


def kernel(x, weight, **_unused):
    x = np.asarray(x, dtype=np.float32)
    weight = np.asarray(weight, dtype=np.float32)
    xt_shards, wp_sb, ind, zz = _host_prep(x, weight)
    nc = _built_program()
    in_maps = [{"xt": xt_shards[c], "wp": wp_sb, "ind": ind, "zz": zz}
               for c in range(N_CORES)]
    res = run_bass_kernel_spmd(nc, in_maps, list(range(N_CORES)))
    outs = []
    for c in range(N_CORES):
        d = np.asarray(res.results[c]["out"]).astype(np.float32)
        d = d.reshape(2, 2, 32, 2, 8, 16, 32)  # [k, v, i, H, ch, w, j]
        # token t = 512H + 256v + 32ch + 16k + w ; element (i, j)
        o = d.transpose(3, 1, 4, 0, 5, 2, 6).reshape(T_CORE, 1024)
        outs.append(o)
    full = np.concatenate(outs, axis=0)                   # [8192, 1024]
    return full.reshape(x.shape[0], x.shape[1], 1024).astype(np.float32)


if __name__ == "__main__":
    rng = np.random.default_rng(0)
    x = rng.standard_normal((4, 2048, D), dtype=np.float32)
    w = (rng.standard_normal((H, D, 64), dtype=np.float32)
         * np.sqrt(2.0 / (D + 64))).astype(np.float32)
    o = kernel(x, w)
    print(o.shape, o.dtype)
